# revision 6
# baseline (speedup 1.0000x reference)
"""Trainium2 Bass kernel for the 2-layer LSTM bar decoder (fp8 DoubleRow).

Model (per bar, 16 bars, shared weights):
  16 steps of: x = [out, emb]; (h0,c0)=LSTMCell0(x); (h1,c1)=LSTMCell1(h0);
  out = softmax(h1 @ W_out.T + b_out)

Strategy (vs fp16 baseline at 1.21 ms):
  - Data-parallel over (bar, batch): each of 8 cores owns 32 batch x 16 bars
    = 512 rows; state transposed [hidden, rows] in SBUF.
  - Gate matmuls in fp8-e4m3 DoubleRow perf mode: one matmul contracts TWO
    128-deep k-tiles (W pairs interleaved on the stationary side, state
    k-tile pairs on the moving side) at the fp16 single-tile cadence ->
    2x Tensor-engine throughput on the dominant 4 gate GEMMs.
  - Scaling: weights x4096, states x32 -> PSUM gates at 2^17; dequant is
    folded into the ACT affine (scale=2^-17 or 2^-18). embpre (emb-part of
    L0 gates + b0, step-invariant) kept bf16 at 2^17 scale, DVE-added into
    PSUM. e4m3 max normal 240 -> 0.044*4096=181 and 32*|state|<=32 are safe.
  - c stored DOUBLED (C=2c), gates as t=tanh(pre/2): sigmoid(x)*y =
    ((t+1)*y)/2 becomes ONE scalar_tensor_tensor per product:
      A=(tf+1)*C; B=(ti+1)*tg; C'=0.5A+B; h2=(to+1)*tanh(0.5C') = 2h.
    The halves fold into ACT scale and W_out/2. No sigmoid affines left.
  - Logits matmul stays fp16 (h2 moving, W_out/2 stationary) - keeps fp8
    noise out of the output softmax (rel err 9.7e-3 vs 2e-2 budget).
  - fp16->fp8 state conversions (x16) on the idle GpSimd engine.
  - softmax: exp (fp16) -> column sums via ones-matmul accumulation on PE,
    reciprocal on DVE, K=1 ones broadcast, e*rec on DVE; final transpose
    on PE and f32 store exactly as the baseline.
"""

import os
import sys

import numpy as np

H = 512
BARS = 16
UNITS = 16
B = 256
NCORES = 8
BPC = B // NCORES  # batch rows per core
R = BARS * BPC  # rows per core (bar-major)
GT = (4 * H) // 128  # gate tiles per layer
KT = H // 128  # k (hidden) tiles
KP = KT // 2  # k-tile pairs (DoubleRow)
RT = R // 128  # row tiles

WS = 4096.0  # fp8 weight scale
XS = 32.0  # fp8 state scale
DQ = 1.0 / (WS * XS)  # PSUM dequant

LAST_EXEC_NS = None

_cache = {}


def _ensure_path():
    for p in ("/opt/trn_rl_repo",):
        if os.path.isdir(p) and p not in sys.path:
            sys.path.insert(0, p)


def _build_nc():
    _ensure_path()
    import concourse.tile as tile
    from concourse import bacc, mybir
    from concourse.masks import make_identity

    f8 = mybir.dt.float8e4
    bf = mybir.dt.bfloat16
    f16 = mybir.dt.float16
    f32 = mybir.dt.float32
    AF = mybir.ActivationFunctionType
    ALU = mybir.AluOpType
    DR = mybir.MatmulPerfMode.DoubleRow

    nc = bacc.Bacc("TRN2")

    # fp8 DoubleRow weights: [kpair, 128, 2, 4H] - [p, r, j, m] = W[m, 256p+128j+r]*WS
    w0a = nc.declare_dram_parameter("w0a", [KP, 128, 2, 4 * H], f8, isOutput=False)
    w0h = nc.declare_dram_parameter("w0h", [KP, 128, 2, 4 * H], f8, isOutput=False)
    w1i = nc.declare_dram_parameter("w1i", [KP, 128, 2, 4 * H], f8, isOutput=False)
    w1h = nc.declare_dram_parameter("w1h", [KP, 128, 2, 4 * H], f8, isOutput=False)
    # fp16 logits weights (W_out/2, transposed), fp16 emb-part weights (x256)
    wo = nc.declare_dram_parameter("wo", [H, H], f16, isOutput=False)
    w0b = nc.declare_dram_parameter("w0b", [H, 4 * H], f16, isOutput=False)
    # biases
    b0s = nc.declare_dram_parameter("b0s", [128, GT], f32, isOutput=False)  # b0*2^17
    b1h = nc.declare_dram_parameter("b1h", [128, GT], f32, isOutput=False)  # b1*0.5
    b1f = nc.declare_dram_parameter("b1f", [128, GT], f32, isOutput=False)  # b1
    bo = nc.declare_dram_parameter("bo", [128, KT], f32, isOutput=False)
    # states: fp8 pairs [kpair, 128, 2, R] (x32), C=2c fp16 [H, R], emb fp16 x512
    h0T8 = nc.declare_dram_parameter("h0T8", [KP, 128, 2, R], f8, isOutput=False)
    h1T8 = nc.declare_dram_parameter("h1T8", [KP, 128, 2, R], f8, isOutput=False)
    oT8 = nc.declare_dram_parameter("oT8", [KP, 128, 2, R], f8, isOutput=False)
    c0T = nc.declare_dram_parameter("c0T", [H, R], f16, isOutput=False)
    c1T = nc.declare_dram_parameter("c1T", [H, R], f16, isOutput=False)
    embT = nc.declare_dram_parameter("embT", [H, R], f16, isOutput=False)
    out = nc.declare_dram_parameter("out", [BPC, BARS * UNITS, H], f32, isOutput=True)

    out_v = out[:, :, :].rearrange("b (bar u) h -> bar u b h", bar=BARS)

    with nc.allow_low_precision(
        reason="fp16 LSTM cell state + fp8 gates, validated against reference"
    ), tile.TileContext(nc) as tc:
        with (
            tc.tile_pool(name="consts", bufs=1) as consts,
            tc.tile_pool(name="wpool", bufs=1) as wpool,
            tc.tile_pool(name="eppool", bufs=1) as eppool,
            tc.tile_pool(name="cpool", bufs=1) as cpool,
            tc.tile_pool(name="hpool", bufs=2) as hpool,
            tc.tile_pool(name="gsb", bufs=1) as gsb,
            tc.tile_pool(name="cellsb", bufs=2) as cellsb,
            tc.tile_pool(name="smx", bufs=2) as smx,
            tc.tile_pool(name="pg", bufs=3, space="PSUM") as pg,
            tc.tile_pool(name="plog", bufs=2, space="PSUM") as plog,
            tc.tile_pool(name="psb", bufs=1, space="PSUM") as psb,
            tc.tile_pool(name="ptr", bufs=2, space="PSUM") as ptr,
        ):
            ident = consts.tile([128, 128], f16, tag="ident")
            make_identity(nc, ident)
            ones_k = consts.tile([128, 1], f16, tag="ones_k")
            nc.vector.memset(ones_k, 1.0)
            ones_m = consts.tile([1, 128], f16, tag="ones_m")
            nc.vector.memset(ones_m, 1.0)

            # warmup: ACT table load (tanh+exp set) + DVE recip table with
            # minimal sync waits (walrus sync-wait limit)
            warm = consts.tile([128, 1], f32, tag="warm")
            nc.scalar.activation(warm[:, :], ones_k[:, :], AF.Tanh)
            nc.scalar.activation(warm[:, :], warm[:, :], AF.Exp)
            nc.vector.reciprocal(warm[:, :], warm[:, :])

            b0s_sb = consts.tile([128, GT], f32, tag="b0s")
            nc.sync.dma_start(out=b0s_sb, in_=b0s[:, :])
            b1h_sb = consts.tile([128, GT], f32, tag="b1h")
            nc.sync.dma_start(out=b1h_sb, in_=b1h[:, :])
            b1f_sb = consts.tile([128, GT], f32, tag="b1f")
            nc.sync.dma_start(out=b1f_sb, in_=b1f[:, :])
            bo_sb = consts.tile([128, KT], f32, tag="bo")
            nc.sync.dma_start(out=bo_sb, in_=bo[:, :])

            def load_pairs(dram, pool, name, free, dtype):
                ts = []
                for p in range(KP):
                    t = pool.tile([128, 2, free], dtype, tag=f"{name}{p}")
                    nc.sync.dma_start(out=t[:, :, :], in_=dram[p, :, :, :])
                    ts.append(t)
                return ts

            def load_ktiles(dram, pool, name, free, dtype):
                ts = []
                for k in range(KT):
                    t = pool.tile([128, free], dtype, tag=f"{name}{k}")
                    nc.sync.dma_start(out=t[:, :], in_=dram[k * 128 : (k + 1) * 128, :])
                    ts.append(t)
                return ts

            w0a_sb = load_pairs(w0a, wpool, "w0a", 4 * H, f8)
            w0h_sb = load_pairs(w0h, wpool, "w0h", 4 * H, f8)
            w1i_sb = load_pairs(w1i, wpool, "w1i", 4 * H, f8)
            w1h_sb = load_pairs(w1h, wpool, "w1h", 4 * H, f8)
            wo_sb = load_ktiles(wo, wpool, "wo", H, f16)

            cur_h0 = load_pairs(h0T8, hpool, "h0p_", R, f8)
            cur_h1 = load_pairs(h1T8, hpool, "h1p_", R, f8)
            cur_o = load_pairs(oT8, hpool, "op_", R, f8)
            c0_sb = load_ktiles(c0T, cpool, "c0_", R, f16)
            c1_sb = load_ktiles(c1T, cpool, "c1_", R, f16)

            # -------- precompute: embpre[gt] = (256*W_ih0_emb)@(512*embT) + b0*2^17
            embpre = []
            with tc.tile_pool(name="prepool", bufs=1) as prepool:
                embT_sb = load_ktiles(embT, prepool, "embT", R, f16)
                w0b_sb = load_ktiles(w0b, prepool, "w0b", 4 * H, f16)
                for gt in range(GT):
                    ps = pg.tile([128, R], f32, tag="g")
                    for k in range(KT):
                        nc.tensor.matmul(
                            ps[:, :],
                            w0b_sb[k][:, gt * 128 : (gt + 1) * 128],
                            embT_sb[k][:, :],
                            start=(k == 0),
                            stop=(k == KT - 1),
                        )
                    ep = eppool.tile([128, R], bf, tag=f"ep{gt}")
                    nc.scalar.activation(
                        ep[:, :], ps[:, :], AF.Identity, bias=b0s_sb[:, gt : gt + 1]
                    )
                    embpre.append(ep)

            # -------- recurrence --------
            def lstm_layer(w_h, h_old, w_x, x_new, emb_add, bias_half, bias_full,
                           c_sb, htag):
                """One layer, transposed layout. Returns h2 (=2h) fp16 k-tiles."""
                tg = [None] * GT
                for gt in range(GT):
                    ps = pg.tile([128, R], f32, tag="g")
                    for p in range(KP):
                        nc.tensor.matmul(
                            ps[:, :],
                            w_h[p][:, :, gt * 128 : (gt + 1) * 128],
                            h_old[p][:, :, :],
                            start=(p == 0),
                            stop=False,
                            perf_mode=DR,
                        )
                    for p in range(KP):
                        nc.tensor.matmul(
                            ps[:, :],
                            w_x[p][:, :, gt * 128 : (gt + 1) * 128],
                            x_new[p][:, :, :],
                            start=False,
                            stop=(p == KP - 1),
                            perf_mode=DR,
                        )
                    if emb_add is not None:
                        nc.vector.tensor_add(ps[:, :], ps[:, :], emb_add[gt][:, :])
                    tgt = gsb.tile([128, R], f16, tag=f"tg{gt}")
                    if 8 <= gt < 12:  # g gate: tanh(pre)
                        if bias_full is not None:
                            nc.scalar.activation(
                                tgt[:, :], ps[:, :], AF.Tanh,
                                bias=bias_full[:, gt : gt + 1], scale=DQ,
                            )
                        else:
                            nc.scalar.activation(tgt[:, :], ps[:, :], AF.Tanh, scale=DQ)
                    else:  # i/f/o: t = tanh(pre/2); sigma = (t+1)/2
                        if bias_half is not None:
                            nc.scalar.activation(
                                tgt[:, :], ps[:, :], AF.Tanh,
                                bias=bias_half[:, gt : gt + 1], scale=DQ * 0.5,
                            )
                        else:
                            nc.scalar.activation(
                                tgt[:, :], ps[:, :], AF.Tanh, scale=DQ * 0.5
                            )
                    tg[gt] = tgt
                h2 = [None] * KT
                for ht in range(KT):
                    ti, tf, tgg, to = tg[ht], tg[4 + ht], tg[8 + ht], tg[12 + ht]
                    # C' = 0.5*(tf+1)*C + (ti+1)*tg   (C = 2c)
                    av = cellsb.tile([128, R], f16, tag=f"av_{ht}")
                    nc.vector.scalar_tensor_tensor(
                        av[:, :], tf[:, :], 1.0, c_sb[ht][:, :], ALU.add, ALU.mult
                    )
                    bv = cellsb.tile([128, R], f16, tag=f"bv_{ht}")
                    nc.vector.scalar_tensor_tensor(
                        bv[:, :], ti[:, :], 1.0, tgg[:, :], ALU.add, ALU.mult
                    )
                    nc.vector.scalar_tensor_tensor(
                        c_sb[ht][:, :], av[:, :], 0.5, bv[:, :], ALU.mult, ALU.add
                    )
                    tch = cellsb.tile([128, R], f16, tag=f"tc_{ht}")
                    nc.scalar.activation(tch[:, :], c_sb[ht][:, :], AF.Tanh, scale=0.5)
                    nh = hpool.tile([128, R], f16, tag=f"{htag}{ht}")
                    nc.vector.scalar_tensor_tensor(
                        nh[:, :], to[:, :], 1.0, tch[:, :], ALU.add, ALU.mult
                    )
                    h2[ht] = nh
                return h2

            def to_fp8_pairs(h2, tag):
                """h2 (=2h) fp16 k-tiles -> fp8 pair tiles (x16 => 32h)."""
                pairs = []
                for p in range(KP):
                    pt = hpool.tile([128, 2, R], f8, tag=f"{tag}{p}")
                    for j in range(2):
                        nc.gpsimd.tensor_scalar_mul(
                            pt[:, j, :], h2[2 * p + j][:, :], 16.0
                        )
                    pairs.append(pt)
                return pairs

            repeat = int(os.environ.get("KREPEAT", "1"))
            for t in list(range(UNITS)) * repeat:
                h2_0 = lstm_layer(
                    w0h_sb, cur_h0, w0a_sb, cur_o, embpre, None, None, c0_sb, "h20_"
                )
                new_h0 = to_fp8_pairs(h2_0, "h0p_")
                h2_1 = lstm_layer(
                    w1h_sb, cur_h1, w1i_sb, new_h0, None, b1h_sb, b1f_sb, c1_sb, "h21_"
                )
                new_h1 = to_fp8_pairs(h2_1, "h1p_")

                # logits (fp16: h2 moving, W_out/2 stationary) -> exp fp16
                e_t = [None] * KT
                for mt in range(KT):
                    ps = plog.tile([128, R], f32, tag="lg")
                    for k in range(KT):
                        nc.tensor.matmul(
                            ps[:, :],
                            wo_sb[k][:, mt * 128 : (mt + 1) * 128],
                            h2_1[k][:, :],
                            start=(k == 0),
                            stop=(k == KT - 1),
                        )
                    et = smx.tile([128, R], f16, tag=f"e{mt}")
                    nc.scalar.activation(
                        et[:, :], ps[:, :], AF.Exp, bias=bo_sb[:, mt : mt + 1]
                    )
                    e_t[mt] = et

                # column sums over hidden via accumulating ones-matmuls
                ps_sum = psb.tile([1, R], f32, tag="sb")
                for mt in range(KT):
                    nc.tensor.matmul(
                        ps_sum[:, :], ones_k[:, :], e_t[mt][:, :],
                        start=(mt == 0), stop=(mt == KT - 1),
                    )
                rec = cellsb.tile([1, R], f16, tag="rec")
                nc.vector.reciprocal(rec[:, :], ps_sum[:, :])
                ps_b = psb.tile([128, R], f32, tag="sb")
                nc.tensor.matmul(ps_b[:, :], ones_m[:, :], rec[:, :], start=True, stop=True)

                new_o16 = [None] * KT
                for mt in range(KT):
                    no = hpool.tile([128, R], f16, tag=f"o16_{mt}")
                    nc.vector.tensor_mul(no[:, :], e_t[mt][:, :], ps_b[:, :])
                    new_o16[mt] = no
                new_o = []
                for p in range(KP):
                    pt = hpool.tile([128, 2, R], f8, tag=f"op_{p}")
                    for j in range(2):
                        nc.gpsimd.tensor_scalar_mul(
                            pt[:, j, :], new_o16[2 * p + j][:, :], 32.0
                        )
                    new_o.append(pt)

                # transpose back to [rows, hidden] and store
                for rt in range(RT):
                    pst = ptr.tile([128, H], f16, tag="tr")
                    for hc in range(KT):
                        nc.tensor.transpose(
                            pst[:, hc * 128 : (hc + 1) * 128],
                            new_o16[hc][:, rt * 128 : (rt + 1) * 128],
                            ident[:, :],
                        )
                    stg = smx.tile([128, H], f32, tag="stg")
                    nc.scalar.activation(stg[:, :], pst[:, :], AF.Identity)
                    nc.sync.dma_start(
                        out=out_v[rt * 4 : (rt + 1) * 4, t, :, :], in_=stg[:, :]
                    )

                cur_h0, cur_h1, cur_o = new_h0, new_h1, new_o

    return nc


def _get_nc():
    if "nc" not in _cache:
        nc = _build_nc()
        if not nc.is_finalized():
            nc.finalize()
        _cache["nc"] = nc
    return _cache["nc"]


def _q8(x, scale):
    import ml_dtypes

    return np.clip(x * scale, -240, 240).astype(ml_dtypes.float8_e4m3)


def _pairs_w(Wmat, scale):
    """[4H, K] weight -> [KP, 128, 2, 4H] fp8 DoubleRow stationary layout."""
    K = Wmat.shape[1]
    wt = np.ascontiguousarray(Wmat.T)  # [K, 4H]
    v = wt.reshape(K // 256, 2, 128, Wmat.shape[0]).transpose(0, 2, 1, 3)
    return _q8(np.ascontiguousarray(v), scale)


def _pairs_x(rows_by_h, scale):
    """[R, H] state -> [KP, 128, 2, R] fp8 moving pair layout (x scale)."""
    xT = np.ascontiguousarray(rows_by_h.T)  # [H, R]
    v = xT.reshape(KP, 2, 128, rows_by_h.shape[0]).transpose(0, 2, 1, 3)
    return _q8(np.ascontiguousarray(v), scale)


def _make_in_maps(inputs):
    x = {k: np.asarray(v) for k, v in inputs.items()}
    W_ih0 = x["W_ih0"].astype(np.float32)
    W_hh0 = x["W_hh0"].astype(np.float32)
    W_ih1 = x["W_ih1"].astype(np.float32)
    W_hh1 = x["W_hh1"].astype(np.float32)
    W_out = x["W_out"].astype(np.float32)
    b0 = (x["b_ih0"] + x["b_hh0"]).astype(np.float32)
    b1 = (x["b_ih1"] + x["b_hh1"]).astype(np.float32)
    b_out = x["b_out"].astype(np.float32)
    emb = x["embedding_C"].astype(np.float32)
    h0 = x["h0"].astype(np.float32)
    c0 = x["c0"].astype(np.float32)
    out0 = x["out0"].astype(np.float32)

    shared = {
        "w0a": _pairs_w(W_ih0[:, :H], WS),
        "w0h": _pairs_w(W_hh0, WS),
        "w1i": _pairs_w(W_ih1, WS),
        "w1h": _pairs_w(W_hh1, WS),
        "wo": np.ascontiguousarray((W_out * 0.5).T).astype(np.float16),
        "w0b": np.ascontiguousarray((W_ih0[:, H:] * 256.0).T).astype(np.float16),
        "b0s": np.ascontiguousarray((b0 * (WS * XS)).reshape(GT, 128).T),
        "b1h": np.ascontiguousarray((b1 * 0.5).reshape(GT, 128).T),
        "b1f": np.ascontiguousarray(b1.reshape(GT, 128).T),
        "bo": np.ascontiguousarray(b_out.reshape(KT, 128).T),
    }

    def t16(rows_by_h, scale):  # [R, H] -> [H, R] fp16 x scale
        return np.ascontiguousarray(rows_by_h.T * scale).astype(np.float16)

    in_maps = []
    for c in range(NCORES):
        bs = slice(c * BPC, (c + 1) * BPC)
        m = dict(shared)
        m["embT"] = t16(np.swapaxes(emb[bs], 0, 1).reshape(R, H), 512.0)
        m["h0T8"] = _pairs_x(h0[:, 0, bs, :].reshape(R, H), XS)
        m["h1T8"] = _pairs_x(h0[:, 1, bs, :].reshape(R, H), XS)
        m["oT8"] = _pairs_x(out0[:, bs, :].reshape(R, H), XS)
        m["c0T"] = t16(c0[:, 0, bs, :].reshape(R, H), 2.0)
        m["c1T"] = t16(c0[:, 1, bs, :].reshape(R, H), 2.0)
        in_maps.append(m)
    return in_maps


def kernel(**inputs):
    global LAST_EXEC_NS
    _ensure_path()
    from concourse.bass_utils import run_bass_kernel_spmd

    in_maps = _make_in_maps(inputs)
    nc = _get_nc()
    trace = bool(os.environ.get("KTRACE"))
    tmpdir = os.environ.get("KTMPDIR") or None
    try:
        res = run_bass_kernel_spmd(
            nc, in_maps, list(range(NCORES)), trace=trace, tmpdir=tmpdir
        )
    except (ImportError, ModuleNotFoundError):
        res = run_bass_kernel_spmd(nc, in_maps, list(range(NCORES)), trace=False)
    _cache["last_res"] = res
    if getattr(res, "exec_time_ns", None):
        LAST_EXEC_NS = res.exec_time_ns

    outs = [np.asarray(res.results[c]["out"], dtype=np.float32) for c in range(NCORES)]
    return np.concatenate(outs, axis=0)


if __name__ == "__main__":
    nc = _get_nc()
    print("built ok")
